# revision 32
# baseline (speedup 1.0000x reference)
"""Causal multi-head attention for Trainium2, sharded over 8 NeuronCores.

Problem: Q,K,V [2, 16, 2048, 128] fp32 -> O [2, 16, 2048, 128] fp32
  scores = (Q @ K^T) / sqrt(128), causal mask, softmax, @ V.

Sharding: the 32 (batch, head) slices are data-parallel; each of the 8
cores computes 4 heads independently (no collectives).

Per-head dataflow on one core (S=2048, D=128, bf16 matmuls, fp32 psum):
  load fp32 (SP-queue DMA) -> DVE cast bf16 -> ONE xbar DMA-transpose per
  tensor (out[:, j, :] = block j transposed, off all compute engines)
  -> PE scores^T per k-block with additive -1e30 diag seeds -> ACT exp
  (scale folded) into a FLAT per-head P^T buffer [128, 17408] so exp runs
  as 12 maximal 1536-col chunks (ACT is the bottleneck engine: 69.6us of
  exp per core is the roofline; everything else is placed to stay under
  it) -> PE mm2 per q-block streams [V | 1] against 128-col P^T slices,
  denominator in the extra column -> DVE reciprocal + scale -> 4-block
  batched stores on the Pool SWDGE queue.

Engine budget per core (4 heads): ACT 69.6us (wall), PE 61.7us,
DVE ~58us, SP-queue ~40us of DMA issue, Pool ~25us. The scores psum uses
2x3 banks double-buffered, mm2 psum 2x1 banks; chunk matmuls are split at
bank boundaries with start=True only on each bank's first toucher (the
hardware clears the whole bank on start).
"""

import math
from contextlib import ExitStack

import numpy as np

N_CORES = 8
B, H, S, D = 2, 16, 2048, 128
HEADS_PER_CORE = (B * H) // N_CORES  # 4
SB = S // 128  # 16 s-blocks per head
SCALE = 1.0 / math.sqrt(128.0)
CHUNK = 1536  # flat P^T columns per exp chunk (3 psum banks)
BANK = 512  # psum bank width in f32
LAG = 2  # mm2 lag in chunks

# flat P^T offsets: off[i] = sum_{j<i} (S - 128j); TOT = 17408
OFF = [2048 * i - 64 * i * (i - 1) for i in range(SB + 1)]
TOT = OFF[SB]
N_CHUNKS = (TOT + CHUNK - 1) // CHUNK  # 12

_CACHE = {}


def _build():
    import concourse.bass as bass
    import concourse.tile as tile
    from concourse import bacc, mybir
    from concourse.masks import make_identity, make_upper_triangular

    f32 = mybir.dt.float32
    bf16 = mybir.dt.bfloat16

    nc = bacc.Bacc("TRN2", num_devices=N_CORES)
    Qd = nc.declare_dram_parameter("Q", [HEADS_PER_CORE, S, D], f32, isOutput=False)
    Kd = nc.declare_dram_parameter("K", [HEADS_PER_CORE, S, D], f32, isOutput=False)
    Vd = nc.declare_dram_parameter("V", [HEADS_PER_CORE, S, D], f32, isOutput=False)
    Od = nc.declare_dram_parameter("O", [HEADS_PER_CORE, S, D], f32, isOutput=True)

    with tile.TileContext(nc) as tc, ExitStack() as ctx:
        const = ctx.enter_context(tc.tile_pool(name="const", bufs=1))
        in_pool = ctx.enter_context(tc.tile_pool(name="inp", bufs=2))
        bf_pool = ctx.enter_context(tc.tile_pool(name="bfp", bufs=2))
        t_pool = ctx.enter_context(tc.tile_pool(name="tp", bufs=2))
        pt_pool = ctx.enter_context(tc.tile_pool(name="ptp", bufs=2))
        o_pool = ctx.enter_context(tc.tile_pool(name="op", bufs=2))
        s_pool = ctx.enter_context(tc.tile_pool(name="sp", bufs=4))
        ps_pool = ctx.enter_context(tc.tile_pool(name="psp", bufs=2, space="PSUM"))
        po_pool = ctx.enter_context(tc.tile_pool(name="pop", bufs=2, space="PSUM"))

        state = {}  # per-head tiles

        def emit_load_qkv(h):
            # loads ride the sync HWDGE queue (full 16-engine striping);
            # the Pool SWDGE queue is ~3x slower and keeps only the stores.
            qn = in_pool.tile([128, SB, D], f32, tag="qn")
            nc.sync.dma_start(qn[:], Qd.ap()[h].rearrange("(p o) d -> p o d", p=128))
            kn = in_pool.tile([128, SB, D], f32, tag="kn")
            nc.sync.dma_start(kn[:], Kd.ap()[h].rearrange("(p o) d -> p o d", p=128))
            vn = in_pool.tile([128, SB, D], f32, tag="vn")
            nc.sync.dma_start(
                vn[:, 0:8, :], Vd.ap()[h].rearrange("(o p) d -> p o d", p=128)[:, 0:8, :]
            )
            nc.sync.dma_start(
                vn[:, 8:SB, :],
                Vd.ap()[h].rearrange("(o p) d -> p o d", p=128)[:, 8:SB, :],
            )
            state[h] = {"qn": qn, "kn": kn, "vn": vn}

        def emit_cast_tr(h, which, part):
            # cast a 4-block slice of Q/K to bf16 on the DVE, then
            # transpose it on the PE — spread over 8 chunk slots so the
            # added PE work per chunk stays under the ACT cadence.
            st = state[h]
            if part == 0:
                tb = bf_pool.tile([128, SB, D], bf16, tag=which + "b")
                st[which + "bb"] = tb
                tt = t_pool.tile([128, 128, SB], bf16, tag=which + "t")
                st[which + "t"] = tt
            tb, tt = st[which + "bb"], st[which + "t"]
            nc.vector.tensor_copy(
                tb[:, 4 * part : 4 * part + 4, :], st[which + "n"][:, 4 * part : 4 * part + 4, :]
            )
            pe_transpose(tt, tb, 4 * part, 4, eye[:])

        def pe_transpose(dst, src_bf, b0, nblk, eye_ap):
            # Transpose 128x128 blocks on the PE. Q/K are loaded row-
            # contiguous (s = 16p + o), so block o holds rows {16p+o}; the
            # copyback scatters columns j-strided into dst [d, p, j],
            # whose flat (p j) order is exactly natural q = 16p + j.
            for g0 in range(b0, b0 + nblk, 8):
                g1 = min(g0 + 8, b0 + nblk)
                trp = ps_pool.tile([128, 1024], bf16, tag="ps", name="trp")
                for j in range(g1 - g0):
                    nc.tensor.transpose(
                        trp[:, 128 * j : 128 * j + 128], src_bf[:, g0 + j, :], eye_ap
                    )
                nc.vector.tensor_copy(
                    dst[:, :, g0:g1],
                    trp[:, 0 : 128 * (g1 - g0)].rearrange(
                        "p (a b) -> p b a", b=128
                    ),
                )

        def emit_cast_v(h):
            st = state[h]
            vp = bf_pool.tile([128, SB, D + 8], bf16, tag="vp")
            nc.vector.tensor_copy(vp[:, :, 0:D], st["vn"][:])
            if h < 2:
                # the ones column survives slot reuse (casts only write 0:D)
                nc.gpsimd.memset(vp[:, :, D : D + 1], 1.0)
            st["vp"] = vp

        def make_mm2(h):
            st = state[h]
            vp = st["vp"]
            pt = st["pt"]

            def emit_mm2(b):
                po = po_pool.tile([128, D + 1], f32, tag="po")
                for i in range(b + 1):
                    c = OFF[i] + 128 * (b - i)
                    nc.tensor.matmul(
                        po[:, 0 : D + 1],
                        lhsT=pt[:, c : c + 128],
                        rhs=vp[:, i, 0 : D + 1],
                        start=(i == 0),
                        stop=(i == b),
                    )
                rec = s_pool.tile([128, 1], f32, tag="rec")
                nc.vector.reciprocal(rec[:], po[:, D : D + 1])
                if b % 4 == 0:
                    st["ob"] = o_pool.tile([128, 4, D], f32, tag="ob", name="ob")
                ob = st["ob"]
                nc.vector.tensor_scalar_mul(ob[:, b % 4, :], po[:, 0:D], rec[:])
                if b % 4 == 3:
                    nc.gpsimd.dma_start(
                        Od.ap()[h, 128 * (b - 3) : 128 * (b + 1), :].rearrange(
                            "(o p) d -> p o d", p=128
                        ),
                        ob[:],
                    )

            return emit_mm2

        # mm2 job queue: (h, b) ready after global chunk index ready_g
        mm2_jobs = []  # built lazily per head
        emitted_mm2 = [0]  # index into mm2_jobs

        def chunk_of(col):
            return col // CHUNK

        def emit_chunk(h, c):
            """mm1 pieces + exp for flat chunk c of head h, then any mm2
            jobs whose data is LAG chunks old."""
            st = state[h]
            if c == 0:
                st["pt"] = pt_pool.tile([128, TOT], bf16, tag="pt", name="pt")
                st["qt2"] = st["qt"][:].rearrange("d p j -> d (p j)")
                st["kt2"] = st["kt"][:].rearrange("d p j -> d (p j)")
                st["mm2"] = make_mm2(h)
                for b in range(SB):
                    mm2_jobs.append((h * N_CHUNKS + chunk_of(OFF[b] + 127), h, b))
            pt, qt2, kt2 = st["pt"], st["qt2"], st["kt2"]

            c0, c1 = CHUNK * c, min(CHUNK * (c + 1), TOT)
            ps = ps_pool.tile([128, CHUNK], f32, tag="ps")
            started = set()  # banks with their start=True toucher emitted

            def bank_pieces(a, b):
                # split flat [a, b) at psum bank boundaries (chunk-relative)
                out = []
                x = a
                while x < b:
                    nb = c0 + ((x - c0) // BANK + 1) * BANK
                    e = min(b, nb)
                    out.append((x, e))
                    x = e
                return out

            # diag seeds first within the chunk so each seeded bank's
            # start=True clear precedes the accumulating score pieces.
            for i in range(SB):
                sa, sb_ = max(OFF[i], c0), min(OFF[i] + 128, c1)
                if sa >= sb_:
                    continue
                for a, b in bank_pieces(sa, sb_):
                    bank = (a - c0) // BANK
                    nc.tensor.matmul(
                        ps[:, a - c0 : b - c0],
                        lhsT=st["eye"],
                        rhs=st["neg_tri"][:, a - OFF[i] : b - OFF[i]],
                        start=bank not in started,
                        stop=False,
                        skip_group_check=True,
                    )
                    started.add(bank)
            # score pieces
            last_in_bank = {}
            pieces = []
            for i in range(SB):
                ia, ib = max(OFF[i], c0), min(OFF[i + 1], c1)
                if ia >= ib:
                    continue
                for a, b in bank_pieces(ia, ib):
                    pieces.append((i, a, b))
                    last_in_bank[(a - c0) // BANK] = (i, a, b)
            for i, a, b in pieces:
                bank = (a - c0) // BANK
                qa = 128 * i + (a - OFF[i])
                nc.tensor.matmul(
                    ps[:, a - c0 : b - c0],
                    lhsT=kt2[:, 128 * i : 128 * i + 128],
                    rhs=qt2[:, qa : qa + (b - a)],
                    start=bank not in started,
                    stop=last_in_bank[bank] == (i, a, b),
                    skip_group_check=True,
                )
                started.add(bank)

            if h == 0 and c == 0:
                # cascade the very first exp so ACT starts on the first
                # filled psum bank instead of waiting for the whole chunk
                for s0 in range(0, c1 - c0, BANK):
                    s1 = min(s0 + BANK, c1 - c0)
                    nc.scalar.activation(
                        pt[:, c0 + s0 : c0 + s1],
                        ps[:, s0:s1],
                        mybir.ActivationFunctionType.Exp,
                        scale=SCALE,
                    )
            else:
                nc.scalar.activation(
                    pt[:, c0:c1],
                    ps[:, 0 : c1 - c0],
                    mybir.ActivationFunctionType.Exp,
                    scale=SCALE,
                )

            # lagged mm2 emission: at most 2 jobs per chunk slot unless
            # the backlog grows, so the per-head tail burst (5 jobs become
            # ready in the last 2 chunks) spreads over the next head's
            # chunks instead of stalling its first mm1s; the last head
            # drains with lag 1 to shorten the kernel tail.
            g = h * N_CHUNKS + c
            lag = 1 if g >= (HEADS_PER_CORE - 1) * N_CHUNKS + 8 else LAG
            budget = 1 if c < 6 else 2
            popped = 0
            while emitted_mm2[0] < len(mm2_jobs):
                ready, bh, b = mm2_jobs[emitted_mm2[0]]
                backlog = g - lag - ready
                if ready > g - lag or (popped >= budget and backlog < 4):
                    break
                state[bh]["mm2"](b)
                emitted_mm2[0] += 1
                popped += 1

        # ---- prologue ----------------------------------------------------
        # ONLY head-0's Q and K load first (full DMA bandwidth to the
        # critical path); V0 follows split; head-1 loads after head-0 prep.
        st0 = state.setdefault(0, {})
        qn0 = in_pool.tile([128, SB, D], f32, tag="qn")
        nc.sync.dma_start(qn0[:], Qd.ap()[0].rearrange("(p o) d -> p o d", p=128))
        kn0 = in_pool.tile([128, SB, D], f32, tag="kn")
        nc.sync.dma_start(kn0[:], Kd.ap()[0].rearrange("(p o) d -> p o d", p=128))
        vn0 = in_pool.tile([128, SB, D], f32, tag="vn")
        nc.sync.dma_start(
            vn0[:, 0:4, :],
            Vd.ap()[0].rearrange("(o p) d -> p o d", p=128)[:, 0:4, :],
        )
        nc.sync.dma_start(
            vn0[:, 4:SB, :],
            Vd.ap()[0].rearrange("(o p) d -> p o d", p=128)[:, 4:SB, :],
        )
        st0.update({"qn": qn0, "kn": kn0, "vn": vn0})

        # consts (built while the prologue loads stream in)
        tri_f = const.tile([128, 128], f32)
        make_upper_triangular(nc, tri_f[:], val=1.0, diag=True)
        neg_tri = const.tile([128, 128], bf16)
        nc.vector.tensor_scalar(
            neg_tri[:], tri_f[:], 1e30, -1e30,
            mybir.AluOpType.mult, mybir.AluOpType.add,
        )
        eye_f = const.tile([128, 128], f32)
        make_identity(nc, eye_f[:])
        eye = const.tile([128, 128], bf16)
        nc.vector.tensor_copy(eye[:], eye_f[:])
        # preload the ACT exp table off the critical path
        warm = const.tile([128, 1], f32)
        nc.scalar.activation(
            warm[:], tri_f[:, 0:1], mybir.ActivationFunctionType.Exp
        )

        # PE p-state warmup: ramp the array while the first loads are in
        # flight so the head-0 transposes run at full clock.
        wrm = const.tile([128, 512], bf16)
        nc.gpsimd.memset(wrm[:], 0.0)
        for _ in range(18):
            psw = ps_pool.tile([128, CHUNK], f32, tag="ps", name="psw")
            nc.tensor.matmul(
                psw[:, 0:512], lhsT=eye[:], rhs=wrm[:], start=True, stop=True
            )

        # head-0 prep, all on the PE: Q first (mm1 chunk 0 needs all of
        # qt and kt), halves pipelined.
        qb0 = bf_pool.tile([128, SB, D], bf16, tag="qb")
        qt0 = t_pool.tile([128, 128, SB], bf16, tag="qt")
        kb0 = bf_pool.tile([128, SB, D], bf16, tag="kb")
        kt0 = t_pool.tile([128, 128, SB], bf16, tag="kt")
        for g in range(4):
            nc.vector.tensor_copy(
                qb0[:, 4 * g : 4 * g + 4, :], qn0[:, 4 * g : 4 * g + 4, :]
            )
            pe_transpose(qt0, qb0, 4 * g, 4, eye[:])
        for g in range(4):
            nc.vector.tensor_copy(
                kb0[:, 4 * g : 4 * g + 4, :], kn0[:, 4 * g : 4 * g + 4, :]
            )
            pe_transpose(kt0, kb0, 4 * g, 4, eye[:])
        st0["qt"], st0["kt"] = qt0, kt0
        st0["eye"], st0["neg_tri"] = eye[:], neg_tri
        # head-0 V cast (Pool) split: first blocks right after V0a lands
        vp0 = bf_pool.tile([128, SB, D + 8], bf16, tag="vp")
        nc.vector.tensor_copy(vp0[:, 0:4, 0:D], vn0[:, 0:4, :])
        nc.gpsimd.memset(vp0[:, :, D : D + 1], 1.0)
        st0["vp"] = vp0
        emit_load_qkv(1)

        # ---- steady state ------------------------------------------------
        for h in range(HEADS_PER_CORE):
            if h > 0:
                state[h]["eye"], state[h]["neg_tri"] = eye[:], neg_tri
            prep0 = 3 if h == 0 else 1
            for c in range(N_CHUNKS):
                emit_chunk(h, c)
                if h == 0 and c == 1:
                    nc.vector.tensor_copy(vp0[:, 4:SB, 0:D], vn0[:, 4:SB, :])
                if h + 1 < HEADS_PER_CORE:
                    if prep0 <= c < prep0 + 4:
                        emit_cast_tr(h + 1, "q", c - prep0)
                    elif prep0 + 4 <= c < prep0 + 8:
                        emit_cast_tr(h + 1, "k", c - prep0 - 4)
                    if c == 2:
                        emit_cast_v(h + 1)
                if h + 2 < HEADS_PER_CORE and c == 5:
                    emit_load_qkv(h + 2)
        # tail flush
        while emitted_mm2[0] < len(mm2_jobs):
            _, bh, b = mm2_jobs[emitted_mm2[0]]
            state[bh]["mm2"](b)
            emitted_mm2[0] += 1

    nc.compile()
    return nc


def _get_nc():
    if "nc" not in _CACHE:
        _CACHE["nc"] = _build()
    return _CACHE["nc"]


def kernel(Q: np.ndarray, K: np.ndarray, V: np.ndarray) -> np.ndarray:
    from concourse.bass_utils import run_bass_kernel_spmd

    Qf = np.ascontiguousarray(np.asarray(Q, dtype=np.float32).reshape(B * H, S, D))
    Kf = np.ascontiguousarray(np.asarray(K, dtype=np.float32).reshape(B * H, S, D))
    Vf = np.ascontiguousarray(np.asarray(V, dtype=np.float32).reshape(B * H, S, D))

    nc = _get_nc()
    in_maps = []
    for c in range(N_CORES):
        sl = slice(c * HEADS_PER_CORE, (c + 1) * HEADS_PER_CORE)
        in_maps.append({"Q": Qf[sl], "K": Kf[sl], "V": Vf[sl]})

    res = run_bass_kernel_spmd(nc, in_maps, core_ids=list(range(N_CORES)))
    out = np.concatenate([res.results[c]["O"] for c in range(N_CORES)], axis=0)
    return out.reshape(B, H, S, D).astype(np.float32)


# revision 33
# speedup vs baseline: 1.1666x; 1.1666x over previous
"""Causal multi-head attention for Trainium2, sharded over 8 NeuronCores.

Problem: Q,K,V [2, 16, 2048, 128] fp32 -> O [2, 16, 2048, 128] fp32
  scores = (Q @ K^T) / sqrt(128), causal mask, softmax, @ V.

Sharding: the 32 (batch, head) slices are data-parallel; each of the 8
cores computes 4 heads independently (no collectives).

Per-head dataflow on one core (S=2048, D=128, bf16 matmuls, fp32 psum):
  load fp32 (SP-queue DMA) -> DVE cast bf16 -> ONE xbar DMA-transpose per
  tensor (out[:, j, :] = block j transposed, off all compute engines)
  -> PE scores^T per k-block with additive -1e30 diag seeds -> ACT exp
  (scale folded) into a FLAT per-head P^T buffer [128, 17408] so exp runs
  as 12 maximal 1536-col chunks (ACT is the bottleneck engine: 69.6us of
  exp per core is the roofline; everything else is placed to stay under
  it) -> PE mm2 per q-block streams [V | 1] against 128-col P^T slices,
  denominator in the extra column -> DVE reciprocal + scale -> 4-block
  batched stores on the Pool SWDGE queue.

Engine budget per core (4 heads): ACT 69.6us (wall), PE 61.7us,
DVE ~58us, SP-queue ~40us of DMA issue, Pool ~25us. The scores psum uses
2x3 banks double-buffered, mm2 psum 2x1 banks; chunk matmuls are split at
bank boundaries with start=True only on each bank's first toucher (the
hardware clears the whole bank on start).
"""

import math
from contextlib import ExitStack

import numpy as np

N_CORES = 8
B, H, S, D = 2, 16, 2048, 128
HEADS_PER_CORE = (B * H) // N_CORES  # 4
SB = S // 128  # 16 s-blocks per head
SCALE = 1.0 / math.sqrt(128.0)
CHUNK = 1536  # flat P^T columns per exp chunk (3 psum banks)
BANK = 512  # psum bank width in f32
LAG = 2  # mm2 lag in chunks

# flat P^T offsets: off[i] = sum_{j<i} (S - 128j); TOT = 17408
OFF = [2048 * i - 64 * i * (i - 1) for i in range(SB + 1)]
TOT = OFF[SB]
N_CHUNKS = (TOT + CHUNK - 1) // CHUNK  # 12

_CACHE = {}


def _build():
    import concourse.bass as bass
    import concourse.tile as tile
    from concourse import bacc, mybir
    from concourse.masks import make_identity, make_upper_triangular

    f32 = mybir.dt.float32
    bf16 = mybir.dt.bfloat16

    nc = bacc.Bacc("TRN2", num_devices=N_CORES)
    Qd = nc.declare_dram_parameter("Q", [HEADS_PER_CORE, S, D], f32, isOutput=False)
    Kd = nc.declare_dram_parameter("K", [HEADS_PER_CORE, S, D], f32, isOutput=False)
    Vd = nc.declare_dram_parameter("V", [HEADS_PER_CORE, S, D], f32, isOutput=False)
    Od = nc.declare_dram_parameter("O", [HEADS_PER_CORE, S, D], f32, isOutput=True)

    with tile.TileContext(nc) as tc, ExitStack() as ctx:
        const = ctx.enter_context(tc.tile_pool(name="const", bufs=1))
        in_pool = ctx.enter_context(tc.tile_pool(name="inp", bufs=2))
        bf_pool = ctx.enter_context(tc.tile_pool(name="bfp", bufs=2))
        t_pool = ctx.enter_context(tc.tile_pool(name="tp", bufs=2))
        pt_pool = ctx.enter_context(tc.tile_pool(name="ptp", bufs=2))
        o_pool = ctx.enter_context(tc.tile_pool(name="op", bufs=2))
        s_pool = ctx.enter_context(tc.tile_pool(name="sp", bufs=4))
        ps_pool = ctx.enter_context(tc.tile_pool(name="psp", bufs=2, space="PSUM"))
        po_pool = ctx.enter_context(tc.tile_pool(name="pop", bufs=2, space="PSUM"))

        state = {}  # per-head tiles

        def emit_load_qkv(h):
            # loads ride the sync HWDGE queue (full 16-engine striping);
            # the Pool SWDGE queue is ~3x slower and keeps only the stores.
            qn = in_pool.tile([128, SB, D], f32, tag="qn")
            nc.sync.dma_start(qn[:], Qd.ap()[h].rearrange("(p o) d -> p o d", p=128))
            kn = in_pool.tile([128, SB, D], f32, tag="kn")
            nc.sync.dma_start(kn[:], Kd.ap()[h].rearrange("(p o) d -> p o d", p=128))
            vn = in_pool.tile([128, SB, D], f32, tag="vn")
            nc.sync.dma_start(
                vn[:, 0:8, :], Vd.ap()[h].rearrange("(o p) d -> p o d", p=128)[:, 0:8, :]
            )
            nc.sync.dma_start(
                vn[:, 8:SB, :],
                Vd.ap()[h].rearrange("(o p) d -> p o d", p=128)[:, 8:SB, :],
            )
            state[h] = {"qn": qn, "kn": kn, "vn": vn}

        def emit_cast_tr(h, which, part):
            # cast a 4-block slice of Q/K to bf16 on the DVE, then
            # transpose it on the PE — spread over 8 chunk slots so the
            # added PE work per chunk stays under the ACT cadence.
            st = state[h]
            if part == 0:
                tb = bf_pool.tile([128, SB, D], bf16, tag=which + "b")
                st[which + "bb"] = tb
                tt = t_pool.tile([128, 128, SB], bf16, tag=which + "t")
                st[which + "t"] = tt
            tb, tt = st[which + "bb"], st[which + "t"]
            nc.vector.tensor_copy(
                tb[:, 4 * part : 4 * part + 4, :], st[which + "n"][:, 4 * part : 4 * part + 4, :]
            )
            pe_transpose(tt, tb, 4 * part, 4, eye[:])

        def pe_transpose(dst, src_bf, b0, nblk, eye_ap):
            # Transpose 128x128 blocks on the PE. Q/K are loaded row-
            # contiguous (s = 16p + o), so block o holds rows {16p+o}; the
            # copyback scatters columns j-strided into dst [d, p, j],
            # whose flat (p j) order is exactly natural q = 16p + j.
            for g0 in range(b0, b0 + nblk, 8):
                g1 = min(g0 + 8, b0 + nblk)
                trp = ps_pool.tile([128, 1024], bf16, tag="ps", name="trp")
                for j in range(g1 - g0):
                    nc.tensor.transpose(
                        trp[:, 128 * j : 128 * j + 128], src_bf[:, g0 + j, :], eye_ap
                    )
                nc.vector.tensor_copy(
                    dst[:, :, g0:g1],
                    trp[:, 0 : 128 * (g1 - g0)].rearrange(
                        "p (a b) -> p b a", b=128
                    ),
                )

        def emit_cast_v(h):
            st = state[h]
            vp = bf_pool.tile([128, SB, D + 8], bf16, tag="vp")
            nc.vector.tensor_copy(vp[:, :, 0:D], st["vn"][:])
            if h < 2:
                # the ones column survives slot reuse (casts only write 0:D)
                nc.gpsimd.memset(vp[:, :, D : D + 1], 1.0)
            st["vp"] = vp

        def make_mm2(h):
            st = state[h]
            vp = st["vp"]
            pt = st["pt"]

            def emit_mm2(b):
                po = po_pool.tile([128, D + 1], f32, tag="po")
                for i in range(b + 1):
                    c = OFF[i] + 128 * (b - i)
                    nc.tensor.matmul(
                        po[:, 0 : D + 1],
                        lhsT=pt[:, c : c + 128],
                        rhs=vp[:, i, 0 : D + 1],
                        start=(i == 0),
                        stop=(i == b),
                    )
                rec = s_pool.tile([128, 1], f32, tag="rec")
                nc.vector.reciprocal(rec[:], po[:, D : D + 1])
                if b % 4 == 0:
                    st["ob"] = o_pool.tile([128, 4, D], f32, tag="ob", name="ob")
                ob = st["ob"]
                nc.vector.tensor_scalar_mul(ob[:, b % 4, :], po[:, 0:D], rec[:])
                if b % 4 == 3:
                    nc.gpsimd.dma_start(
                        Od.ap()[h, 128 * (b - 3) : 128 * (b + 1), :].rearrange(
                            "(o p) d -> p o d", p=128
                        ),
                        ob[:],
                    )

            return emit_mm2

        # mm2 job queue: (h, b) ready after global chunk index ready_g
        mm2_jobs = []  # built lazily per head
        emitted_mm2 = [0]  # index into mm2_jobs

        def chunk_of(col):
            return col // CHUNK

        def emit_chunk(h, c):
            """mm1 pieces + exp for flat chunk c of head h, then any mm2
            jobs whose data is LAG chunks old."""
            st = state[h]
            if c == 0:
                st["pt"] = pt_pool.tile([128, TOT], bf16, tag="pt", name="pt")
                st["qt2"] = st["qt"][:].rearrange("d p j -> d (p j)")
                st["kt2"] = st["kt"][:].rearrange("d p j -> d (p j)")
                st["mm2"] = make_mm2(h)
                for b in range(SB):
                    mm2_jobs.append((h * N_CHUNKS + chunk_of(OFF[b] + 127), h, b))
            pt, qt2, kt2 = st["pt"], st["qt2"], st["kt2"]

            c0, c1 = CHUNK * c, min(CHUNK * (c + 1), TOT)
            ps = ps_pool.tile([128, CHUNK], f32, tag="ps")
            started = set()  # banks with their start=True toucher emitted

            def bank_pieces(a, b):
                # split flat [a, b) at psum bank boundaries (chunk-relative)
                out = []
                x = a
                while x < b:
                    nb = c0 + ((x - c0) // BANK + 1) * BANK
                    e = min(b, nb)
                    out.append((x, e))
                    x = e
                return out

            # diag seeds first within the chunk so each seeded bank's
            # start=True clear precedes the accumulating score pieces.
            for i in range(SB):
                sa, sb_ = max(OFF[i], c0), min(OFF[i] + 128, c1)
                if sa >= sb_:
                    continue
                for a, b in bank_pieces(sa, sb_):
                    bank = (a - c0) // BANK
                    nc.tensor.matmul(
                        ps[:, a - c0 : b - c0],
                        lhsT=st["eye"],
                        rhs=st["neg_tri"][:, a - OFF[i] : b - OFF[i]],
                        start=bank not in started,
                        stop=False,
                        skip_group_check=True,
                    )
                    started.add(bank)
            # score pieces
            last_in_bank = {}
            pieces = []
            for i in range(SB):
                ia, ib = max(OFF[i], c0), min(OFF[i + 1], c1)
                if ia >= ib:
                    continue
                for a, b in bank_pieces(ia, ib):
                    pieces.append((i, a, b))
                    last_in_bank[(a - c0) // BANK] = (i, a, b)
            for i, a, b in pieces:
                bank = (a - c0) // BANK
                qa = 128 * i + (a - OFF[i])
                nc.tensor.matmul(
                    ps[:, a - c0 : b - c0],
                    lhsT=kt2[:, 128 * i : 128 * i + 128],
                    rhs=qt2[:, qa : qa + (b - a)],
                    start=bank not in started,
                    stop=last_in_bank[bank] == (i, a, b),
                    skip_group_check=True,
                )
                started.add(bank)

            nc.scalar.activation(
                pt[:, c0:c1],
                ps[:, 0 : c1 - c0],
                mybir.ActivationFunctionType.Exp,
                scale=SCALE,
            )

            # lagged mm2 emission: at most 2 jobs per chunk slot unless
            # the backlog grows, so the per-head tail burst (5 jobs become
            # ready in the last 2 chunks) spreads over the next head's
            # chunks instead of stalling its first mm1s; the last head
            # drains with lag 1 to shorten the kernel tail.
            g = h * N_CHUNKS + c
            lag = 1 if g >= (HEADS_PER_CORE - 1) * N_CHUNKS + 8 else LAG
            budget = 1 if c < 6 else 2
            popped = 0
            while emitted_mm2[0] < len(mm2_jobs):
                ready, bh, b = mm2_jobs[emitted_mm2[0]]
                backlog = g - lag - ready
                if ready > g - lag or (popped >= budget and backlog < 4):
                    break
                state[bh]["mm2"](b)
                emitted_mm2[0] += 1
                popped += 1

        # ---- prologue ----------------------------------------------------
        # ONLY head-0's Q and K load first (full DMA bandwidth to the
        # critical path); V0 follows split; head-1 loads after head-0 prep.
        st0 = state.setdefault(0, {})
        qn0 = in_pool.tile([128, SB, D], f32, tag="qn")
        nc.sync.dma_start(qn0[:], Qd.ap()[0].rearrange("(p o) d -> p o d", p=128))
        kn0 = in_pool.tile([128, SB, D], f32, tag="kn")
        nc.sync.dma_start(kn0[:], Kd.ap()[0].rearrange("(p o) d -> p o d", p=128))
        vn0 = in_pool.tile([128, SB, D], f32, tag="vn")
        nc.sync.dma_start(
            vn0[:, 0:4, :],
            Vd.ap()[0].rearrange("(o p) d -> p o d", p=128)[:, 0:4, :],
        )
        nc.sync.dma_start(
            vn0[:, 4:SB, :],
            Vd.ap()[0].rearrange("(o p) d -> p o d", p=128)[:, 4:SB, :],
        )
        st0.update({"qn": qn0, "kn": kn0, "vn": vn0})

        # consts (built while the prologue loads stream in)
        tri_f = const.tile([128, 128], f32)
        make_upper_triangular(nc, tri_f[:], val=1.0, diag=True)
        neg_tri = const.tile([128, 128], bf16)
        nc.vector.tensor_scalar(
            neg_tri[:], tri_f[:], 1e30, -1e30,
            mybir.AluOpType.mult, mybir.AluOpType.add,
        )
        eye_f = const.tile([128, 128], f32)
        make_identity(nc, eye_f[:])
        eye = const.tile([128, 128], bf16)
        nc.vector.tensor_copy(eye[:], eye_f[:])
        # preload the ACT exp table off the critical path
        warm = const.tile([128, 1], f32)
        nc.scalar.activation(
            warm[:], tri_f[:, 0:1], mybir.ActivationFunctionType.Exp
        )

        # PE p-state warmup: ramp the array while the first loads are in
        # flight so the head-0 transposes run at full clock.
        wrm = const.tile([128, 512], bf16)
        nc.gpsimd.memset(wrm[:], 0.0)
        for _ in range(18):
            psw = ps_pool.tile([128, CHUNK], f32, tag="ps", name="psw")
            nc.tensor.matmul(
                psw[:, 0:512], lhsT=eye[:], rhs=wrm[:], start=True, stop=True
            )

        # head-0 prep, all on the PE: Q first (mm1 chunk 0 needs all of
        # qt and kt), halves pipelined.
        qb0 = bf_pool.tile([128, SB, D], bf16, tag="qb")
        qt0 = t_pool.tile([128, 128, SB], bf16, tag="qt")
        kb0 = bf_pool.tile([128, SB, D], bf16, tag="kb")
        kt0 = t_pool.tile([128, 128, SB], bf16, tag="kt")
        for g in range(4):
            nc.vector.tensor_copy(
                qb0[:, 4 * g : 4 * g + 4, :], qn0[:, 4 * g : 4 * g + 4, :]
            )
            pe_transpose(qt0, qb0, 4 * g, 4, eye[:])
        for g in range(4):
            nc.vector.tensor_copy(
                kb0[:, 4 * g : 4 * g + 4, :], kn0[:, 4 * g : 4 * g + 4, :]
            )
            pe_transpose(kt0, kb0, 4 * g, 4, eye[:])
        st0["qt"], st0["kt"] = qt0, kt0
        st0["eye"], st0["neg_tri"] = eye[:], neg_tri
        # head-0 V cast (Pool) split: first blocks right after V0a lands
        vp0 = bf_pool.tile([128, SB, D + 8], bf16, tag="vp")
        nc.vector.tensor_copy(vp0[:, 0:4, 0:D], vn0[:, 0:4, :])
        nc.gpsimd.memset(vp0[:, :, D : D + 1], 1.0)
        st0["vp"] = vp0
        emit_load_qkv(1)

        # ---- steady state ------------------------------------------------
        for h in range(HEADS_PER_CORE):
            if h > 0:
                state[h]["eye"], state[h]["neg_tri"] = eye[:], neg_tri
            prep0 = 3 if h == 0 else 1
            for c in range(N_CHUNKS):
                emit_chunk(h, c)
                if h == 0 and c == 1:
                    nc.vector.tensor_copy(vp0[:, 4:SB, 0:D], vn0[:, 4:SB, :])
                if h + 1 < HEADS_PER_CORE:
                    if prep0 <= c < prep0 + 4:
                        emit_cast_tr(h + 1, "q", c - prep0)
                    elif prep0 + 4 <= c < prep0 + 8:
                        emit_cast_tr(h + 1, "k", c - prep0 - 4)
                    if c == 2:
                        emit_cast_v(h + 1)
                if h + 2 < HEADS_PER_CORE and c == 5:
                    emit_load_qkv(h + 2)
        # tail flush
        while emitted_mm2[0] < len(mm2_jobs):
            _, bh, b = mm2_jobs[emitted_mm2[0]]
            state[bh]["mm2"](b)
            emitted_mm2[0] += 1

    nc.compile()
    return nc


def _get_nc():
    if "nc" not in _CACHE:
        _CACHE["nc"] = _build()
    return _CACHE["nc"]


def kernel(Q: np.ndarray, K: np.ndarray, V: np.ndarray) -> np.ndarray:
    from concourse.bass_utils import run_bass_kernel_spmd

    Qf = np.ascontiguousarray(np.asarray(Q, dtype=np.float32).reshape(B * H, S, D))
    Kf = np.ascontiguousarray(np.asarray(K, dtype=np.float32).reshape(B * H, S, D))
    Vf = np.ascontiguousarray(np.asarray(V, dtype=np.float32).reshape(B * H, S, D))

    nc = _get_nc()
    in_maps = []
    for c in range(N_CORES):
        sl = slice(c * HEADS_PER_CORE, (c + 1) * HEADS_PER_CORE)
        in_maps.append({"Q": Qf[sl], "K": Kf[sl], "V": Vf[sl]})

    res = run_bass_kernel_spmd(nc, in_maps, core_ids=list(range(N_CORES)))
    out = np.concatenate([res.results[c]["O"] for c in range(N_CORES)], axis=0)
    return out.reshape(B, H, S, D).astype(np.float32)


# revision 34
# speedup vs baseline: 1.1897x; 1.0198x over previous
"""Causal multi-head attention for Trainium2, sharded over 8 NeuronCores.

Problem: Q,K,V [2, 16, 2048, 128] fp32 -> O [2, 16, 2048, 128] fp32
  scores = (Q @ K^T) / sqrt(128), causal mask, softmax, @ V.

Sharding: the 32 (batch, head) slices are data-parallel; each of the 8
cores computes 4 heads independently (no collectives).

Per-head dataflow on one core (S=2048, D=128, bf16 matmuls, fp32 psum):
  load fp32 (SP-queue DMA) -> DVE cast bf16 -> ONE xbar DMA-transpose per
  tensor (out[:, j, :] = block j transposed, off all compute engines)
  -> PE scores^T per k-block with additive -1e30 diag seeds -> ACT exp
  (scale folded) into a FLAT per-head P^T buffer [128, 17408] so exp runs
  as 12 maximal 1536-col chunks (ACT is the bottleneck engine: 69.6us of
  exp per core is the roofline; everything else is placed to stay under
  it) -> PE mm2 per q-block streams [V | 1] against 128-col P^T slices,
  denominator in the extra column -> DVE reciprocal + scale -> 4-block
  batched stores on the Pool SWDGE queue.

Engine budget per core (4 heads): ACT 69.6us (wall), PE 61.7us,
DVE ~58us, SP-queue ~40us of DMA issue, Pool ~25us. The scores psum uses
2x3 banks double-buffered, mm2 psum 2x1 banks; chunk matmuls are split at
bank boundaries with start=True only on each bank's first toucher (the
hardware clears the whole bank on start).
"""

import math
from contextlib import ExitStack

import numpy as np

N_CORES = 8
B, H, S, D = 2, 16, 2048, 128
HEADS_PER_CORE = (B * H) // N_CORES  # 4
SB = S // 128  # 16 s-blocks per head
SCALE = 1.0 / math.sqrt(128.0)
CHUNK = 1536  # flat P^T columns per exp chunk (3 psum banks)
BANK = 512  # psum bank width in f32
LAG = 2  # mm2 lag in chunks

# flat P^T offsets: off[i] = sum_{j<i} (S - 128j); TOT = 17408
OFF = [2048 * i - 64 * i * (i - 1) for i in range(SB + 1)]
TOT = OFF[SB]
N_CHUNKS = (TOT + CHUNK - 1) // CHUNK  # 12

_CACHE = {}


def _build():
    import concourse.bass as bass
    import concourse.tile as tile
    from concourse import bacc, mybir
    from concourse.masks import make_identity, make_upper_triangular

    f32 = mybir.dt.float32
    bf16 = mybir.dt.bfloat16

    nc = bacc.Bacc("TRN2", num_devices=N_CORES)
    Qd = nc.declare_dram_parameter("Q", [HEADS_PER_CORE, S, D], f32, isOutput=False)
    Kd = nc.declare_dram_parameter("K", [HEADS_PER_CORE, S, D], f32, isOutput=False)
    Vd = nc.declare_dram_parameter("V", [HEADS_PER_CORE, S, D], f32, isOutput=False)
    Od = nc.declare_dram_parameter("O", [HEADS_PER_CORE, S, D], f32, isOutput=True)

    with tile.TileContext(nc) as tc, ExitStack() as ctx:
        const = ctx.enter_context(tc.tile_pool(name="const", bufs=1))
        in_pool = ctx.enter_context(tc.tile_pool(name="inp", bufs=2))
        bf_pool = ctx.enter_context(tc.tile_pool(name="bfp", bufs=2))
        t_pool = ctx.enter_context(tc.tile_pool(name="tp", bufs=2))
        pt_pool = ctx.enter_context(tc.tile_pool(name="ptp", bufs=2))
        o_pool = ctx.enter_context(tc.tile_pool(name="op", bufs=2))
        s_pool = ctx.enter_context(tc.tile_pool(name="sp", bufs=4))
        ps_pool = ctx.enter_context(tc.tile_pool(name="psp", bufs=2, space="PSUM"))
        po_pool = ctx.enter_context(tc.tile_pool(name="pop", bufs=2, space="PSUM"))

        state = {}  # per-head tiles

        def emit_load_qkv(h):
            # loads ride the sync HWDGE queue (full 16-engine striping);
            # the Pool SWDGE queue is ~3x slower and keeps only the stores.
            qn = in_pool.tile([128, SB, D], f32, tag="qn")
            nc.sync.dma_start(qn[:], Qd.ap()[h].rearrange("(p o) d -> p o d", p=128))
            kn = in_pool.tile([128, SB, D], f32, tag="kn")
            nc.sync.dma_start(kn[:], Kd.ap()[h].rearrange("(p o) d -> p o d", p=128))
            vn = in_pool.tile([128, SB, D], f32, tag="vn")
            nc.sync.dma_start(
                vn[:, 0:8, :], Vd.ap()[h].rearrange("(o p) d -> p o d", p=128)[:, 0:8, :]
            )
            nc.sync.dma_start(
                vn[:, 8:SB, :],
                Vd.ap()[h].rearrange("(o p) d -> p o d", p=128)[:, 8:SB, :],
            )
            state[h] = {"qn": qn, "kn": kn, "vn": vn}

        def emit_cast_tr(h, which, part):
            # cast a 4-block slice of Q/K to bf16 on the DVE, then
            # transpose it on the PE — spread over 8 chunk slots so the
            # added PE work per chunk stays under the ACT cadence.
            st = state[h]
            if part == 0:
                tb = bf_pool.tile([128, SB, D], bf16, tag=which + "b")
                st[which + "bb"] = tb
                tt = t_pool.tile([128, 128, SB], bf16, tag=which + "t")
                st[which + "t"] = tt
            tb, tt = st[which + "bb"], st[which + "t"]
            nc.vector.tensor_copy(
                tb[:, 4 * part : 4 * part + 4, :], st[which + "n"][:, 4 * part : 4 * part + 4, :]
            )
            pe_transpose(tt, tb, 4 * part, 4, eye[:])

        def pe_transpose(dst, src_bf, b0, nblk, eye_ap):
            # Transpose 128x128 blocks on the PE. Q/K are loaded row-
            # contiguous (s = 16p + o), so block o holds rows {16p+o}; the
            # copyback scatters columns j-strided into dst [d, p, j],
            # whose flat (p j) order is exactly natural q = 16p + j.
            for g0 in range(b0, b0 + nblk, 8):
                g1 = min(g0 + 8, b0 + nblk)
                trp = ps_pool.tile([128, 1024], bf16, tag="ps", name="trp")
                for j in range(g1 - g0):
                    nc.tensor.transpose(
                        trp[:, 128 * j : 128 * j + 128], src_bf[:, g0 + j, :], eye_ap
                    )
                nc.vector.tensor_copy(
                    dst[:, :, g0:g1],
                    trp[:, 0 : 128 * (g1 - g0)].rearrange(
                        "p (a b) -> p b a", b=128
                    ),
                )

        def emit_cast_v(h):
            st = state[h]
            vp = bf_pool.tile([128, SB, D + 8], bf16, tag="vp")
            nc.vector.tensor_copy(vp[:, :, 0:D], st["vn"][:])
            if h < 2:
                # the ones column survives slot reuse (casts only write 0:D)
                nc.gpsimd.memset(vp[:, :, D : D + 1], 1.0)
            st["vp"] = vp

        def make_mm2(h):
            st = state[h]
            vp = st["vp"]
            pt = st["pt"]

            def emit_mm2(b):
                po = po_pool.tile([128, D + 1], f32, tag="po")
                for i in range(b + 1):
                    c = OFF[i] + 128 * (b - i)
                    nc.tensor.matmul(
                        po[:, 0 : D + 1],
                        lhsT=pt[:, c : c + 128],
                        rhs=vp[:, i, 0 : D + 1],
                        start=(i == 0),
                        stop=(i == b),
                    )
                rec = s_pool.tile([128, 1], f32, tag="rec")
                nc.vector.reciprocal(rec[:], po[:, D : D + 1])
                if b % 4 == 0:
                    st["ob"] = o_pool.tile([128, 4, D], f32, tag="ob", name="ob")
                ob = st["ob"]
                nc.vector.tensor_scalar_mul(ob[:, b % 4, :], po[:, 0:D], rec[:])
                if b % 4 == 3:
                    nc.gpsimd.dma_start(
                        Od.ap()[h, 128 * (b - 3) : 128 * (b + 1), :].rearrange(
                            "(o p) d -> p o d", p=128
                        ),
                        ob[:],
                    )

            return emit_mm2

        # mm2 job queue: (h, b) ready after global chunk index ready_g
        mm2_jobs = []  # built lazily per head
        emitted_mm2 = [0]  # index into mm2_jobs

        def chunk_of(col):
            return col // CHUNK

        def emit_chunk(h, c):
            """mm1 pieces + exp for flat chunk c of head h, then any mm2
            jobs whose data is LAG chunks old."""
            st = state[h]
            if c == 0:
                st["pt"] = pt_pool.tile([128, TOT], bf16, tag="pt", name="pt")
                st["qt2"] = st["qt"][:].rearrange("d p j -> d (p j)")
                st["kt2"] = st["kt"][:].rearrange("d p j -> d (p j)")
                st["mm2"] = make_mm2(h)
                for b in range(SB):
                    mm2_jobs.append((h * N_CHUNKS + chunk_of(OFF[b] + 127), h, b))
            pt, qt2, kt2 = st["pt"], st["qt2"], st["kt2"]

            c0, c1 = CHUNK * c, min(CHUNK * (c + 1), TOT)
            ps = ps_pool.tile([128, CHUNK], f32, tag="ps")
            started = set()  # banks with their start=True toucher emitted

            def bank_pieces(a, b):
                # split flat [a, b) at psum bank boundaries (chunk-relative)
                out = []
                x = a
                while x < b:
                    nb = c0 + ((x - c0) // BANK + 1) * BANK
                    e = min(b, nb)
                    out.append((x, e))
                    x = e
                return out

            # diag seeds first within the chunk so each seeded bank's
            # start=True clear precedes the accumulating score pieces.
            for i in range(SB):
                sa, sb_ = max(OFF[i], c0), min(OFF[i] + 128, c1)
                if sa >= sb_:
                    continue
                for a, b in bank_pieces(sa, sb_):
                    bank = (a - c0) // BANK
                    nc.tensor.matmul(
                        ps[:, a - c0 : b - c0],
                        lhsT=st["eye"],
                        rhs=st["neg_tri"][:, a - OFF[i] : b - OFF[i]],
                        start=bank not in started,
                        stop=False,
                        skip_group_check=True,
                    )
                    started.add(bank)
            # score pieces
            last_in_bank = {}
            pieces = []
            for i in range(SB):
                ia, ib = max(OFF[i], c0), min(OFF[i + 1], c1)
                if ia >= ib:
                    continue
                for a, b in bank_pieces(ia, ib):
                    pieces.append((i, a, b))
                    last_in_bank[(a - c0) // BANK] = (i, a, b)
            for i, a, b in pieces:
                bank = (a - c0) // BANK
                qa = 128 * i + (a - OFF[i])
                nc.tensor.matmul(
                    ps[:, a - c0 : b - c0],
                    lhsT=kt2[:, 128 * i : 128 * i + 128],
                    rhs=qt2[:, qa : qa + (b - a)],
                    start=bank not in started,
                    stop=last_in_bank[bank] == (i, a, b),
                    skip_group_check=True,
                )
                started.add(bank)

            if h == 0 and c == 0:
                # cascade the very first exp so ACT starts on the first
                # filled psum bank instead of waiting for the whole chunk
                for s0 in range(0, c1 - c0, BANK):
                    s1 = min(s0 + BANK, c1 - c0)
                    nc.scalar.activation(
                        pt[:, c0 + s0 : c0 + s1],
                        ps[:, s0:s1],
                        mybir.ActivationFunctionType.Exp,
                        scale=SCALE,
                    )
            else:
                nc.scalar.activation(
                    pt[:, c0:c1],
                    ps[:, 0 : c1 - c0],
                    mybir.ActivationFunctionType.Exp,
                    scale=SCALE,
                )

            # lagged mm2 emission: at most 2 jobs per chunk slot unless
            # the backlog grows, so the per-head tail burst (5 jobs become
            # ready in the last 2 chunks) spreads over the next head's
            # chunks instead of stalling its first mm1s; the last head
            # drains with lag 1 to shorten the kernel tail.
            g = h * N_CHUNKS + c
            lag = 1 if g >= (HEADS_PER_CORE - 1) * N_CHUNKS + 8 else LAG
            budget = 1 if c < 6 else 2
            popped = 0
            while emitted_mm2[0] < len(mm2_jobs):
                ready, bh, b = mm2_jobs[emitted_mm2[0]]
                backlog = g - lag - ready
                if ready > g - lag or (popped >= budget and backlog < 4):
                    break
                state[bh]["mm2"](b)
                emitted_mm2[0] += 1
                popped += 1

        # ---- prologue ----------------------------------------------------
        # ONLY head-0's Q and K load first (full DMA bandwidth to the
        # critical path); V0 follows split; head-1 loads after head-0 prep.
        st0 = state.setdefault(0, {})
        qn0 = in_pool.tile([128, SB, D], f32, tag="qn")
        nc.sync.dma_start(qn0[:], Qd.ap()[0].rearrange("(p o) d -> p o d", p=128))
        kn0 = in_pool.tile([128, SB, D], f32, tag="kn")
        nc.sync.dma_start(kn0[:], Kd.ap()[0].rearrange("(p o) d -> p o d", p=128))
        vn0 = in_pool.tile([128, SB, D], f32, tag="vn")
        nc.sync.dma_start(
            vn0[:, 0:4, :],
            Vd.ap()[0].rearrange("(o p) d -> p o d", p=128)[:, 0:4, :],
        )
        nc.sync.dma_start(
            vn0[:, 4:SB, :],
            Vd.ap()[0].rearrange("(o p) d -> p o d", p=128)[:, 4:SB, :],
        )
        st0.update({"qn": qn0, "kn": kn0, "vn": vn0})

        # consts (built while the prologue loads stream in)
        tri_f = const.tile([128, 128], f32)
        make_upper_triangular(nc, tri_f[:], val=1.0, diag=True)
        neg_tri = const.tile([128, 128], bf16)
        nc.vector.tensor_scalar(
            neg_tri[:], tri_f[:], 1e30, -1e30,
            mybir.AluOpType.mult, mybir.AluOpType.add,
        )
        eye_f = const.tile([128, 128], f32)
        make_identity(nc, eye_f[:])
        eye = const.tile([128, 128], bf16)
        nc.vector.tensor_copy(eye[:], eye_f[:])
        # preload the ACT exp table off the critical path
        warm = const.tile([128, 1], f32)
        nc.scalar.activation(
            warm[:], tri_f[:, 0:1], mybir.ActivationFunctionType.Exp
        )

        # PE p-state warmup: ramp the array while the first loads are in
        # flight so the head-0 transposes run at full clock.
        wrm = const.tile([128, 512], bf16)
        nc.gpsimd.memset(wrm[:], 0.0)
        for _ in range(18):
            psw = ps_pool.tile([128, CHUNK], f32, tag="ps", name="psw")
            nc.tensor.matmul(
                psw[:, 0:512], lhsT=eye[:], rhs=wrm[:], start=True, stop=True
            )

        # head-0 prep, all on the PE: Q first (mm1 chunk 0 needs all of
        # qt and kt), halves pipelined.
        qb0 = bf_pool.tile([128, SB, D], bf16, tag="qb")
        qt0 = t_pool.tile([128, 128, SB], bf16, tag="qt")
        kb0 = bf_pool.tile([128, SB, D], bf16, tag="kb")
        kt0 = t_pool.tile([128, 128, SB], bf16, tag="kt")
        for g in range(4):
            nc.vector.tensor_copy(
                qb0[:, 4 * g : 4 * g + 4, :], qn0[:, 4 * g : 4 * g + 4, :]
            )
            pe_transpose(qt0, qb0, 4 * g, 4, eye[:])
        for g in range(4):
            nc.vector.tensor_copy(
                kb0[:, 4 * g : 4 * g + 4, :], kn0[:, 4 * g : 4 * g + 4, :]
            )
            pe_transpose(kt0, kb0, 4 * g, 4, eye[:])
        st0["qt"], st0["kt"] = qt0, kt0
        st0["eye"], st0["neg_tri"] = eye[:], neg_tri
        # head-0 V cast (Pool) split: first blocks right after V0a lands
        vp0 = bf_pool.tile([128, SB, D + 8], bf16, tag="vp")
        nc.vector.tensor_copy(vp0[:, 0:4, 0:D], vn0[:, 0:4, :])
        nc.gpsimd.memset(vp0[:, :, D : D + 1], 1.0)
        st0["vp"] = vp0
        emit_load_qkv(1)

        # ---- steady state ------------------------------------------------
        for h in range(HEADS_PER_CORE):
            if h > 0:
                state[h]["eye"], state[h]["neg_tri"] = eye[:], neg_tri
            prep0 = 3 if h == 0 else 1
            for c in range(N_CHUNKS):
                emit_chunk(h, c)
                if h == 0 and c == 1:
                    nc.vector.tensor_copy(vp0[:, 4:SB, 0:D], vn0[:, 4:SB, :])
                if h + 1 < HEADS_PER_CORE:
                    if prep0 <= c < prep0 + 4:
                        emit_cast_tr(h + 1, "q", c - prep0)
                    elif prep0 + 4 <= c < prep0 + 8:
                        emit_cast_tr(h + 1, "k", c - prep0 - 4)
                    if c == 2:
                        emit_cast_v(h + 1)
                if h + 2 < HEADS_PER_CORE and c == 5:
                    emit_load_qkv(h + 2)
        # tail flush
        while emitted_mm2[0] < len(mm2_jobs):
            _, bh, b = mm2_jobs[emitted_mm2[0]]
            state[bh]["mm2"](b)
            emitted_mm2[0] += 1

    nc.compile()
    return nc


def _get_nc():
    if "nc" not in _CACHE:
        _CACHE["nc"] = _build()
    return _CACHE["nc"]


def kernel(Q: np.ndarray, K: np.ndarray, V: np.ndarray) -> np.ndarray:
    from concourse.bass_utils import run_bass_kernel_spmd

    Qf = np.ascontiguousarray(np.asarray(Q, dtype=np.float32).reshape(B * H, S, D))
    Kf = np.ascontiguousarray(np.asarray(K, dtype=np.float32).reshape(B * H, S, D))
    Vf = np.ascontiguousarray(np.asarray(V, dtype=np.float32).reshape(B * H, S, D))

    nc = _get_nc()
    in_maps = []
    for c in range(N_CORES):
        sl = slice(c * HEADS_PER_CORE, (c + 1) * HEADS_PER_CORE)
        in_maps.append({"Q": Qf[sl], "K": Kf[sl], "V": Vf[sl]})

    res = run_bass_kernel_spmd(nc, in_maps, core_ids=list(range(N_CORES)))
    out = np.concatenate([res.results[c]["O"] for c in range(N_CORES)], axis=0)
    return out.reshape(B, H, S, D).astype(np.float32)
